# revision 49
# baseline (speedup 1.0000x reference)
"""Trainium2 Bass kernel for nn_AttnAdapter: GQA attention with RoPE,
region-based enhance/suppress score scaling, causal mask, o_proj.

Sharding: tensor-parallel over heads across 8 NeuronCores. Core d holds
q-heads 4d..4d+3 (wq rows), kv-head d (wk/wv rows), and wo columns
512d..512(d+1). Each core computes a full [S, D] partial of the output;
the host sums the 8 partials (the TP all-reduce, done at unshard time).

v2: all matmuls in bf16 (fp32 PSUM accumulate), weights resident in
SBUF (single load), exp batched over [128,1024] PSUM pairs, region
scale folded into pre-scaled K columns, reciprocal on DVE, all PSUM
evictions on DVE, batched DMAs.
"""

import math

import numpy as np

# ---- problem constants (hardcoded; kernel.py must be self-contained) ----
S = 2048          # sequence length
D = 4096          # model dim
HD = 128          # head dim
NCORES = 8
QH = 4            # q heads per core
SYS_LEN, IMG_LEN = 35, 576
BOUND = SYS_LEN + IMG_LEN          # 611
ENH, SUP = 1.5, 0.5
ROPE_BASE = 10000.0

J = 4             # sq tiles of 512
NSK = 16          # sk tiles of 128
DCH = 32          # D chunks of 128
G = 4             # d-groups of 8 chunks
GD = 8            # d-chunks per group

_CACHE = {}


def _bf16():
    import ml_dtypes
    return ml_dtypes.bfloat16


def _host_constants():
    bf16 = _bf16()
    inv_freq = 1.0 / (ROPE_BASE ** (np.arange(0, HD, 2, dtype=np.float32) / HD))
    pos = np.arange(S, dtype=np.float32)
    freqs = pos[:, None] * inv_freq[None, :]              # [S, 64]
    emb = np.concatenate([freqs, freqs], axis=-1)         # [S, 128]
    cosT = np.ascontiguousarray(np.cos(emb).T.astype(np.float32))  # [128, S]
    # rotate_half is a partition swap plus a sign flip: rot(q)[0:64] =
    # -q[64:128], rot(q)[64:128] = q[0:64]. The swap rides two SBUF->SBUF
    # DMAs and the sign lives in sinF (rows 0:64 negated), so no PE matmul
    # or PSUM is needed for RoPE at all.
    sinF = np.sin(emb).T.astype(np.float32)
    sinF[:HD // 2] = -sinF[:HD // 2]
    sinF = np.ascontiguousarray(sinF)

    ident = np.eye(HD, dtype=np.float32)

    # Diagonal-tile causal masks, T layout [sk 128, sq 512]:
    # tile (i=4j+delta, j): valid (keep) iff sq >= sk  <=>  f >= 128*delta + p
    masks = np.zeros((HD, 4 * 512), dtype=np.float32)
    p = np.arange(128)[:, None]
    f = np.arange(512)[None, :]
    for delta in range(4):
        masks[:, delta * 512:(delta + 1) * 512] = (f >= 128 * delta + p)

    kpos = np.arange(S)
    key_scale = np.where(kpos < SYS_LEN, SUP,
                         np.where(kpos < BOUND, ENH, 1.0)).astype(np.float32)
    # per-partition scale per sk-tile (j=1 partial-row DVE path)
    ksT = np.ascontiguousarray(key_scale[:5 * 128].reshape(5, 128).T)  # [128, 5]
    # per-key-column broadcast (for pre-scaling krot columns, j>=2 path)
    ksB = np.ascontiguousarray(
        np.broadcast_to(key_scale[:640][None, :], (HD, 640)))          # [128, 640]

    return dict(
        cosT=cosT, sinF=sinF,
        ident=ident.astype(bf16),
        masks=masks.astype(bf16), ksT=ksT, ksB=ksB.astype(bf16),
        ones_mat=np.ones((HD, HD), dtype=bf16),
    )


def _build_bass():
    import concourse.bass as bass
    import concourse.mybir as mybir
    from concourse.tile import TileContext
    from contextlib import ExitStack

    f32 = mybir.dt.float32
    f32r = mybir.dt.float32r
    bf = mybir.dt.bfloat16

    nc = bass.Bass()
    # x packed per (j, g): [128, GD*512] with [p, dd*512+f] = x.T[128(8g+dd)+p, 512j+f]
    xp = nc.dram_tensor("xp", [J, G, 128, GD * 512], bf, kind="ExternalInput")
    # wq packed per g: [128, GD*512] with [p, dd*512+f] = wq_d.T[128(8g+dd)+p, f]
    wqp = nc.dram_tensor("wqp", [G, 128, GD * 512], bf, kind="ExternalInput")
    # wk/wv packed per g: [128, GD*256], cols dd*256+0:128 = wkT, 128:256 = wvT
    wkvp = nc.dram_tensor("wkvp", [G, 128, GD * 256], bf, kind="ExternalInput")
    # wo packed: [128, 4*4096] with [p, h*4096+f] = wo[:, core].T[128h+p, f]
    wop = nc.dram_tensor("wop", [128, QH * D], bf, kind="ExternalInput")
    cosT_d = nc.dram_tensor("cosT", [HD, S], f32, kind="ExternalInput")
    sinF_d = nc.dram_tensor("sinF", [HD, S], f32, kind="ExternalInput")
    ident_d = nc.dram_tensor("ident", [HD, HD], bf, kind="ExternalInput")
    masks_d = nc.dram_tensor("masks", [HD, 4 * 512], bf, kind="ExternalInput")
    ksT_d = nc.dram_tensor("ksT", [HD, 5], f32, kind="ExternalInput")
    ksB_d = nc.dram_tensor("ksB", [HD, 640], bf, kind="ExternalInput")
    onesm_d = nc.dram_tensor("ones_mat", [HD, HD], bf, kind="ExternalInput")
    # out.T tiles: out_t[j, cq, p, cb*512+f] = out.T[128*(4cq+cb)+p, 512j+f]
    out = nc.dram_tensor("out", [J, DCH // 4, 128, 4 * 512], bf,
                         kind="ExternalOutput")

    EXP = mybir.ActivationFunctionType.Exp

    with TileContext(nc) as tc, ExitStack() as ctx:
        # ---- resident weights (single load, used all phases) ----
        # Order matters for startup latency: the first matmuls need only
        # wq/wkv chunk g=0 and x(j=0,g=0); everything else trails behind.
        wsb = ctx.enter_context(tc.tile_pool(name="wsb", bufs=1))
        wq_sb = wsb.tile([128, DCH * 512], bf)
        wkv_sb = wsb.tile([128, DCH * 256], bf)
        xgp = ctx.enter_context(tc.tile_pool(name="xg", bufs=3))
        xpre = {}
        xt0 = xgp.tile([128, GD * 512], bf, tag="xt", name="xt0")
        # Fine-grained first slices so the first matmuls (which need only
        # dd=0 of g=0) start after ~320KB of DMA instead of ~2MB, spread
        # over the two fast HWDGE queues: wq+x interleaved on SP, wkv (and
        # later the consts) on ACT. The gpsimd SWDGE queue measures only
        # ~40-70 GB/s, so it carries no phase-A-critical traffic.
        # 2+3 slices per stream: 512-col for dd 0-1 (smallest first dep),
        # 1024-col after — few enough descriptors that the ~600ns-per-
        # descriptor programming on the queues keeps ahead of consumption
        cuts = [0, 512, 1024, 2048, 3072, 4096]
        for a, b_ in zip(cuts, cuts[1:]):
            nc.sync.dma_start(wq_sb[:, a:b_], wqp[0][:, a:b_])
            nc.sync.dma_start(xt0[:, a:b_], xp[0, 0][:, a:b_])
            nc.scalar.dma_start(wkv_sb[:, a // 2:b_ // 2],
                                wkvp[0][:, a // 2:b_ // 2])
        xpre[(0, 0)] = xt0
        for g in range(1, G):
            # wq g1-3 rides the ACT queue: the SP queue is reserved for the
            # x stream (16.8MB) so x tiles never queue behind weights
            nc.scalar.dma_start(wq_sb[:, g * GD * 512:(g + 1) * GD * 512],
                                wqp[g])
            nc.scalar.dma_start(wkv_sb[:, g * GD * 256:(g + 1) * GD * 256],
                                wkvp[g])
            if g <= 2:
                xt_n = xgp.tile([128, GD * 512], bf, tag="xt", name=f"xt{g}")
                nc.sync.dma_start(xt_n[:], xp[0, g])
                xpre[(0, g)] = xt_n

        # const tiles allocated here; their DMAs are deferred until after
        # j=0's x tiles are queued so phase A's first block never starves
        const = ctx.enter_context(tc.tile_pool(name="const", bufs=1))
        cosT = const.tile([HD, S], f32)
        sinF = const.tile([HD, S], f32)
        ident = const.tile([HD, HD], bf)
        masks = const.tile([HD, 4 * 512], bf)
        ksT = const.tile([HD, 5], f32)
        ksB = const.tile([HD, 640], bf)
        ones_mat = const.tile([HD, HD], bf)
        wo_sb = const.tile([128, QH * D], bf)

        def emit_const_dmas():
            # consts ride the ACT HWDGE queue (done programming its wkv
            # slices by then, and ACT's compute work starts much later)
            nc.scalar.dma_start(cosT[:], cosT_d[:, :])
            nc.scalar.dma_start(sinF[:], sinF_d[:, :])
            nc.scalar.dma_start(ident[:], ident_d[:, :])
            nc.scalar.dma_start(masks[:], masks_d[:, :])
            nc.scalar.dma_start(ksT[:], ksT_d[:, :])
            nc.scalar.dma_start(ksB[:], ksB_d[:, :])
            nc.scalar.dma_start(ones_mat[:], onesm_d[:, :])
            nc.scalar.dma_start(wo_sb[:], wop[:, :])

        # j=1's q-rope intermediates live here (not in phase A's stage pool)
        # so closing the A pools never waits on the deferred GPSIMD chain
        ropeL = ctx.enter_context(tc.tile_pool(name="ropeL", bufs=2))
        persist = ctx.enter_context(tc.tile_pool(name="persist", bufs=1))
        qrot = [persist.tile([HD, S], bf, name=f"qrot{m}") for m in range(QH)]
        krot = persist.tile([HD, S], bf)
        krot_s = persist.tile([HD, 640], bf)     # region-scaled K cols (keys<640)
        vnat = persist.tile([HD, NSK * HD], bf)  # tile i at cols i*128
        attn = [persist.tile([HD, S], bf, name=f"attn{h}") for h in range(QH)]

        # ---------------- Phase A: projections + RoPE + V transpose --------
        # Phase A runs j in [0, 2, 3, 1] so region E's blocks (2, 3) get
        # their RoPE outputs mid-A; the last epilogue (j=1) feeds only
        # region L, ~70us later.
        with tc.tile_pool(name="accp", bufs=1, space="PSUM") as accp, \
             tc.tile_pool(name="vtp", bufs=2, space="PSUM") as vtp, \
             tc.tile_pool(name="stage", bufs=2) as stage:
            for j in (0, 2, 3, 1):
                sq = slice(j * 512, (j + 1) * 512)
                accs = [accp.tile([128, 512], f32, name=f"acc{m}") for m in range(6)]
                for g in range(G):
                    if (j, g) in xpre:
                        xt = xpre.pop((j, g))
                    else:
                        xt = xgp.tile([128, GD * 512], bf, tag="xt")
                        nc.sync.dma_start(xt[:], xp[j, g])
                        if j == 0 and g == G - 1:
                            emit_const_dmas()
                    def emit_proj(m, dd):
                        d = g * GD + dd
                        xs = xt[:, dd * 512:(dd + 1) * 512]
                        st = (d == 0)
                        sp_ = (d == DCH - 1)
                        if m < QH:
                            w = wq_sb[:, d * 512 + m * 128:d * 512 + (m + 1) * 128]
                            nc.tensor.matmul(accs[m][:], w, xs,
                                             start=st, stop=sp_)
                        else:
                            kb = d * 256 + (m - QH) * 128
                            nc.tensor.matmul(accs[m][:], wkv_sb[:, kb:kb + 128],
                                             xs, start=st, stop=sp_)

                    if j == 0 and g == 0:
                        # dd-major: consumption tracks the fine-grained
                        # lead-in DMA slices
                        for dd in range(GD):
                            for m in range(6):
                                emit_proj(m, dd)
                    else:
                        # m-major within the group, k/v passes first: each
                        # acc's accumulation stops progressively earlier than
                        # the j boundary, matching the epilogue's eviction
                        # order (k, v, q0..q3), so the copies drain inside
                        # the matmul phase instead of stalling PE at the
                        # boundary
                        for m in (4, 5, 0, 1, 2, 3):
                            for dd in range(GD):
                                emit_proj(m, dd)

                # RoPE epilogue: evict acc to f32 SBUF (full precision), swap
                # halves via two SBUF->SBUF DMAs, then dst = q*cos + swap*sinF
                # on DVE/GPSIMD. No PE matmul, no PSUM bank. The swap DMAs
                # ride the ACT HWDGE queue (its const/weight traffic is done
                # by the first epilogue); the j=1 k-rope uses the SP queue
                # (idle at A end) since region E's pair p2 needs it soonest.
                def emit_rope(m, copy_eng, mul_eng, dma_eng):
                    dst = qrot[m][:, sq] if m < QH else krot[:, sq]
                    q_sb = stage.tile([128, 512], f32, tag="q_sb", bufs=2)
                    if copy_eng == "act":
                        nc.scalar.copy(q_sb[:], accs[m][:])
                    else:
                        nc.vector.tensor_copy(q_sb[:], accs[m][:])
                    rp = stage.tile([128, 512], f32, tag="rp", bufs=2)
                    dma_eng.dma_start(rp[0:64, :], q_sb[64:128, :])
                    dma_eng.dma_start(rp[64:128, :], q_sb[0:64, :])
                    eng = nc.vector if mul_eng == "dve" else nc.gpsimd
                    t1 = stage.tile([128, 512], f32, tag="t1", bufs=2)
                    eng.tensor_mul(t1[:], q_sb[:], cosT[:, sq])
                    t2 = stage.tile([128, 512], f32, tag="t2", bufs=2)
                    eng.tensor_mul(t2[:], rp[:], sinF[:, sq])
                    eng.tensor_add(dst, t1[:], t2[:])

                def emit_vtrans():
                    v_sb = stage.tile([128, 512], bf, tag="v_sb")
                    nc.scalar.copy(v_sb[:], accs[5][:])
                    vt_ps = vtp.tile([128, 512], bf, tag="vt_ps")
                    for b in range(4):
                        nc.tensor.transpose(vt_ps[:, b * 128:(b + 1) * 128],
                                            v_sb[:, b * 128:(b + 1) * 128],
                                            ident[:])
                    nc.vector.tensor_copy(
                        vnat[:, (4 * j) * 128:(4 * j + 4) * 128], vt_ps[:])

                if j != 1:
                    emit_rope(4, "act", "dve", nc.scalar)
                    emit_vtrans()
                    for m in range(QH):
                        emit_rope(m, "act", "dve", nc.scalar)
                    if j == 0:
                        nc.vector.tensor_mul(krot_s[:, 0:512], krot[:, 0:512],
                                             ksB[:, 0:512])
                else:
                    # Last epilogue (j=1): all six acc evictions go FIRST
                    # (1 ACT + 3 DVE q-copies, then v + k on ACT) so region
                    # E's PSUM pools, which reuse these banks, unblock within
                    # ~2us. k's rope chain stays on DVE (E's pair p2 needs
                    # krot tiles 4-7 in ~5us); the q ropes run on GPSIMD with
                    # tiles from ropeL — their only consumer is region L.
                    q_sbs = {}
                    for m, ceng in ((0, "act"), (1, "dve"),
                                    (2, "dve"), (3, "dve")):
                        qs = ropeL.tile([128, 512], f32, tag=f"q{m}", bufs=1,
                                        name=f"q_sb1_{m}")
                        if ceng == "act":
                            nc.scalar.copy(qs[:], accs[m][:])
                        else:
                            nc.vector.tensor_copy(qs[:], accs[m][:])
                        q_sbs[m] = qs
                    emit_vtrans()
                    emit_rope(4, "act", "dve", nc.sync)
                    nc.vector.tensor_mul(krot_s[:, 512:640], krot[:, 512:640],
                                         ksB[:, 512:640])
                    for m in range(QH):
                        dst = qrot[m][:, sq]
                        qs = q_sbs[m]
                        rp = ropeL.tile([128, 512], f32, tag="rp", bufs=1, name="rp1")
                        nc.scalar.dma_start(rp[0:64, :], qs[64:128, :])
                        nc.scalar.dma_start(rp[64:128, :], qs[0:64, :])
                        t1 = ropeL.tile([128, 512], f32, tag="t1", bufs=1, name="t11")
                        nc.gpsimd.tensor_mul(t1[:], qs[:], cosT[:, sq])
                        t2 = ropeL.tile([128, 512], f32, tag="t2", bufs=1, name="t21")
                        nc.gpsimd.tensor_mul(t2[:], rp[:], sinF[:, sq])
                        nc.gpsimd.tensor_add(dst, t1[:], t2[:])

        # ------------- Phase B+C: attention fused with o_proj --------------
        # Two regions. Region E: block j=3 alone (no o_proj available yet)
        # processed as WIDE PAIRS — two key tiles share one [128,1024] PSUM
        # scores tile and one exp instruction, halving the per-exp fixed
        # overhead that would otherwise make ACT the pacer; its finalizes
        # run on DVE (reciprocal) to stay off the ACT FIFO. Region L: blocks
        # 1, 2, 0 narrow with o_proj C chunks of finished blocks emitted
        # between the scores prefetch and the tail, so PE has independent
        # work in the exp shadow; finalizes on ACT (slack there).
        with tc.tile_pool(name="att_w", bufs=3) as att_w, \
             tc.tile_pool(name="att_sb", bufs=4) as att_sb, \
             tc.tile_pool(name="nrm", bufs=2) as nrm, \
             tc.tile_pool(name="ost", bufs=2) as ost:
            state = {}
            osb_state = {}

            def kt_for(j, i):
                # scaled K for full-region rows (j>=2), keys<640
                if j >= 2 and i < 5:
                    return krot_s[:, i * 128:(i + 1) * 128]
                return krot[:, i * 128:(i + 1) * 128]

            def emit_finalize(j, h, on_dve=False):
                sq = slice(j * 512, (j + 1) * 512)
                acc_av, acc_dn = state.pop((j, h))
                # acc_dn is already partition-broadcast (all-ones stationary)
                rb_sb = nrm.tile([128, 512], f32, tag="rb_sb")
                if on_dve:
                    nc.vector.reciprocal(rb_sb[:], acc_dn[:])
                else:
                    lrec = nrm.tile([128, 512], f32, tag="lrec")
                    nc.scalar.activation(lrec[:], acc_dn[:],
                                         mybir.ActivationFunctionType.Ln)
                    nc.scalar.activation(rb_sb[:], lrec[:], EXP, scale=-1.0)
                nc.vector.tensor_mul(attn[h][:, sq], acc_av[:], rb_sb[:])

            # ---------------- Region E: blocks 2 then 3, wide pairs --------
            with tc.tile_pool(name="spw", bufs=2, space="PSUM") as spw, \
                 tc.tile_pool(name="avpE", bufs=2, space="PSUM") as avpE, \
                 tc.tile_pool(name="dnpE", bufs=2, space="PSUM") as dnpE:
                seqE = [(jE, h, p) for jE in (2, 3) for h in range(QH)
                        for p in range((4 * jE + 4) // 2)]
                # B(2) h0's pair p2 reads krot_s[512:640], produced by the
                # j=1 k-rope chain ~4us after phase A ends; run p3 (plain
                # krot, ready ~1us earlier) ahead of it to shrink the wait
                seqE[2], seqE[3] = seqE[3], seqE[2]

                def emit_scores_w(jE, h, p):
                    sq0 = jE * 512
                    s_ps = spw.tile([128, 1024], f32, tag="sw")
                    for u in (0, 1):
                        i = 2 * p + u
                        delta = i - 4 * jE
                        c0 = 128 * delta if delta >= 1 else 0
                        nc.tensor.matmul(
                            s_ps[:, u * 512 + c0:(u + 1) * 512],
                            kt_for(jE, i),
                            qrot[h][:, sq0 + c0:sq0 + 512],
                            start=True, stop=True)
                    return s_ps

                def emit_tail_w(jE, h, p, s_ps):
                    npairs = (4 * jE + 4) // 2
                    if p == 0:
                        state[(jE, h)] = (
                            avpE.tile([128, 512], f32, tag="av", name="aE"),
                            dnpE.tile([128, 512], f32, tag="dn", name="dE"))
                    acc_av, acc_dn = state[(jE, h)]
                    e_w = att_w.tile([128, 1024], bf, tag="ew")
                    d0 = 2 * p - 4 * jE
                    if d0 >= 0:
                        # diagonal pair: per-tile exps over the live windows
                        # (the gap holds stale PSUM; exp'ing it would NaN via
                        # inf*0) + the usual 0/1 mask multiplies
                        for u in (0, 1):
                            d = d0 + u
                            c0 = 128 * d if d >= 1 else 0
                            lo = u * 512 + c0
                            nc.scalar.activation(e_w[:, lo:(u + 1) * 512],
                                                 s_ps[:, lo:(u + 1) * 512],
                                                 EXP)
                            nc.vector.tensor_mul(
                                e_w[:, lo:(u + 1) * 512],
                                e_w[:, lo:(u + 1) * 512],
                                masks[:, d * 512 + c0:(d + 1) * 512])
                    else:
                        nc.scalar.activation(e_w[:], s_ps[:], EXP)
                    for u in (0, 1):
                        i = 2 * p + u
                        d = d0 + u
                        c0 = 128 * d if d >= 1 else 0
                        st = (p == 0 and u == 0)
                        sp_l = (p == npairs - 1 and u == 1)
                        eb = e_w[:, u * 512 + c0:(u + 1) * 512]
                        nc.tensor.matmul(acc_dn[:, c0:512], ones_mat[:], eb,
                                         start=st, stop=sp_l)
                        nc.tensor.matmul(acc_av[:, c0:512],
                                         vnat[:, i * 128:(i + 1) * 128],
                                         eb, start=st, stop=sp_l)

                pendingE = None
                sw_cur = emit_scores_w(*seqE[0])
                for k, (jE, h, p) in enumerate(seqE):
                    nxt = seqE[k + 1] if k + 1 < len(seqE) else None
                    sw_next = emit_scores_w(*nxt) if nxt is not None else None
                    emit_tail_w(jE, h, p, sw_cur)
                    sw_cur = sw_next
                    if pendingE is not None and pendingE != (jE, h):
                        emit_finalize(*pendingE, on_dve=True)
                        pendingE = None
                    if p == (4 * jE + 4) // 2 - 1:
                        pendingE = (jE, h)
                # flush inside the region: the acc tiles must be consumed
                # before their PSUM banks are re-pooled by region L. ACT is
                # free at the region boundary and beats DVE's slow
                # reciprocal, so the last block's C unblocks sooner in L.
                if pendingE is not None:
                    emit_finalize(*pendingE, on_dve=False)

            # ---------------- Region L: blocks 1, 2, 0 + o_proj ------------
            # op pool first: pool->bank assignment follows creation order, so
            # op lands on region E's earliest-freed banks (spw slot A) and
            # the first C chunks don't wait on E's last finalizes
            with tc.tile_pool(name="op", bufs=2, space="PSUM") as op, \
                 tc.tile_pool(name="sp", bufs=2, space="PSUM") as sp, \
                 tc.tile_pool(name="avp", bufs=2, space="PSUM") as avp, \
                 tc.tile_pool(name="dnp", bufs=2, space="PSUM") as dnp:
                border = [1, 0]
                # C chunks interleaved: C(2) into B(1), C(3) into B(0),
                # C(1) + C(0) trailing as pure PE work
                cmap = {1: 2, 0: 3}
                # last block processes head 0 last so trailing C(0) chunks
                # (whose o_proj accumulation ends with head 0) wait minimally
                horder = {j: list(range(QH)) for j in range(J)}
                horder[border[-1]] = [1, 2, 3, 0]
                b_seq = []
                attach = {}   # index into b_seq -> list of C items
                for j in border:
                    bi = [("B", j, h, i)
                          for h in horder[j] for i in range(4 * j + 4)]
                    base = len(b_seq)
                    b_seq.extend(bi)
                    jc = cmap[j]
                    cc = [("C", jc, c) for c in range(DCH)]
                    nb = len(bi)
                    ci = 0
                    for idx in range(nb):
                        # hold C back a few items so the source block's last
                        # finalize lands before its chunks are consumed
                        while (ci < len(cc) and idx >= 3
                               and ci + 1 <= (idx - 2) * len(cc) / (nb - 3)):
                            attach.setdefault(base + idx, []).append(cc[ci])
                            ci += 1
                    attach.setdefault(base + nb - 1, []).extend(cc[ci:])
                tail_c = [("C", 1, c) for c in range(DCH)]
                tail_c += [("C", 0, c) for c in range(DCH)]
                b_next = {}
                for a, b in zip(b_seq, b_seq[1:]):
                    b_next[a] = b

                def emit_scores(el):
                    _, j, h, i = el
                    delta = i - 4 * j
                    c0 = 128 * delta if delta >= 1 else 0  # trimmed live cols
                    sq0 = j * 512
                    s_ps = sp.tile([128, 512], f32, tag="s")
                    nc.tensor.matmul(s_ps[:, c0:512], kt_for(j, i),
                                     qrot[h][:, sq0 + c0:sq0 + 512],
                                     start=True, stop=True)
                    if j == 1 and i < 5:
                        # partial region rows: queries 611.. are cols 99..
                        nc.vector.tensor_scalar_mul(
                            s_ps[:, 99:512], s_ps[:, 99:512], ksT[:, i:i + 1])
                    return s_ps

                def emit_tail(el, s_ps):
                    _, j, h, i = el
                    ni = 4 * j + 4
                    delta = i - 4 * j
                    c0 = 128 * delta if delta >= 1 else 0
                    if i == 0:
                        state[(j, h)] = (
                            avp.tile([128, 512], f32, tag="av", name="acc_av"),
                            dnp.tile([128, 512], f32, tag="dn", name="acc_dn"))
                    acc_av, acc_dn = state[(j, h)]
                    e_sb = att_sb.tile([128, 512], bf, tag="e")
                    nc.scalar.activation(e_sb[:, c0:512], s_ps[:, c0:512], EXP)
                    if delta >= 0:
                        nc.vector.tensor_mul(
                            e_sb[:, c0:512], e_sb[:, c0:512],
                            masks[:, delta * 512 + c0:(delta + 1) * 512])
                    st = (i == 0)
                    sp_l = (i == ni - 1)
                    eb = e_sb[:, c0:512]
                    # all-ones stationary: every out partition gets the
                    # key-sum, i.e. the denominator arrives pre-broadcast
                    # (same PE cost — cost is rows, not cols)
                    nc.tensor.matmul(acc_dn[:, c0:512], ones_mat[:], eb,
                                     start=st, stop=sp_l)
                    nc.tensor.matmul(acc_av[:, c0:512],
                                     vnat[:, i * 128:(i + 1) * 128],
                                     eb, start=st, stop=sp_l)

                def emit_cchunk(jc, c):
                    sq = slice(jc * 512, (jc + 1) * 512)
                    if c % 4 == 0:
                        osb_state[jc] = ost.tile([128, 4 * 512], bf,
                                                 tag="o_sb", name="o_sb")
                    o_sb = osb_state[jc]
                    cb = c % 4
                    o_ps = op.tile([128, 512], f32, tag="o")
                    for hi, h in enumerate([1, 2, 3, 0]):
                        nc.tensor.matmul(
                            o_ps[:],
                            wo_sb[:, h * D + c * 128:h * D + (c + 1) * 128],
                            attn[h][:, sq], start=(hi == 0),
                            stop=(hi == QH - 1))
                    # evictions stay off ACT: the strict-FIFO ACT queue
                    # would delay the exps that pace the pipeline
                    nc.vector.tensor_copy(
                        o_sb[:, cb * 512:(cb + 1) * 512], o_ps[:])
                    # two half-DMAs: first half on the (slow, otherwise idle)
                    # gpsimd SWDGE queue, last half on the fast SP queue so
                    # the final transfer at kernel end drains quickly
                    if cb == 1:
                        nc.gpsimd.dma_start(out[jc, c // 4][:, 0:1024],
                                            o_sb[:, 0:1024])
                    elif cb == 3:
                        nc.sync.dma_start(out[jc, c // 4][:, 1024:2048],
                                          o_sb[:, 1024:2048])

                s2_cur = emit_scores(b_seq[0])
                pending = None
                for bidx, el in enumerate(b_seq):
                    nxt = b_next.get(el)
                    s2_next = emit_scores(nxt) if nxt is not None else None
                    for cel in attach.get(bidx, ()):
                        # a C chunk reads all four attn heads of its block:
                        # any pending finalize for that block must land first
                        if pending is not None and pending[0] == cel[1]:
                            emit_finalize(*pending)
                            pending = None
                        emit_cchunk(cel[1], cel[2])
                    emit_tail(el, s2_cur)
                    s2_cur = s2_next
                    if pending is not None and pending != (el[1], el[2]):
                        emit_finalize(*pending)
                        pending = None
                    if el[3] == 4 * el[1] + 3:   # last key tile of (j, h)
                        pending = (el[1], el[2])
                for cel in tail_c:
                    if pending is not None and pending[0] == cel[1]:
                        emit_finalize(*pending)
                        pending = None
                    emit_cchunk(cel[1], cel[2])
                if pending is not None:
                    emit_finalize(*pending)

    # Split multi-wait instructions onto standalone EventSemaphore insts.
    import bass_rust
    bass_rust.generate_event_semaphores(nc)
    return nc


def _get_compiled():
    if "nc" not in _CACHE:
        _CACHE["nc"] = _build_bass()
        _CACHE["const"] = _host_constants()
    return _CACHE["nc"], _CACHE["const"]


def kernel(hidden_states, wq, wk, wv, wo, _trace=False):
    from concourse.bass_utils import run_bass_kernel_spmd

    nc, cst = _get_compiled()
    bf16 = _bf16()

    x = np.asarray(hidden_states, dtype=np.float32).reshape(S, D)
    xT = x.T                                             # [D, S]
    # xp[j, g, p, dd*512+f] = xT[128*(8g+dd)+p, 512j+f]
    t = xT.reshape(G, GD, 128, J, 512)                   # [g, dd, p, j, f]
    xpk = np.ascontiguousarray(
        t.transpose(3, 0, 2, 1, 4).reshape(J, G, 128, GD * 512)).astype(bf16)
    wq = np.asarray(wq, dtype=np.float32)
    wk = np.asarray(wk, dtype=np.float32)
    wv = np.asarray(wv, dtype=np.float32)
    wo = np.asarray(wo, dtype=np.float32)
    scale = 1.0 / math.sqrt(HD)

    in_maps = []
    for d in range(NCORES):
        wq_d = wq[d * QH * HD:(d + 1) * QH * HD] * scale      # [512, D]
        wqT = wq_d.T                                          # [4096, 512]
        wqpk = np.ascontiguousarray(
            wqT.reshape(G, GD, 128, 512).transpose(0, 2, 1, 3)
            .reshape(G, 128, GD * 512)).astype(bf16)
        wkT = wk[d * HD:(d + 1) * HD].T.reshape(G, GD, 128, 128)
        wvT = wv[d * HD:(d + 1) * HD].T.reshape(G, GD, 128, 128)
        kv = np.concatenate([wkT, wvT], axis=-1)              # [G, GD, 128, 256]
        wkvpk = np.ascontiguousarray(
            kv.transpose(0, 2, 1, 3).reshape(G, 128, GD * 256)).astype(bf16)
        woT = wo[:, d * QH * HD:(d + 1) * QH * HD].T          # [512, 4096]
        wopk = np.ascontiguousarray(
            woT.reshape(QH, 128, D).transpose(1, 0, 2)
            .reshape(128, QH * D)).astype(bf16)
        in_maps.append({
            "xp": xpk,
            "wqp": wqpk,
            "wkvp": wkvpk,
            "wop": wopk,
            "cosT": cst["cosT"], "sinF": cst["sinF"],
            "ident": cst["ident"],
            "masks": cst["masks"], "ksT": cst["ksT"], "ksB": cst["ksB"],
            "ones_mat": cst["ones_mat"],
        })

    res = run_bass_kernel_spmd(nc, in_maps, core_ids=list(range(NCORES)),
                               trace=_trace)
    acc = res.results[0]["out"].astype(np.float32)
    for d in range(1, NCORES):
        acc += res.results[d]["out"].astype(np.float32)
    # out_t[j, cq, p, cb*512+f] = out.T[128*(4cq+cb)+p, 512j+f]
    acc = acc.reshape(J, DCH // 4, 128, 4, 512)          # [j, cq, p, cb, f]
    outp = acc.transpose(0, 4, 1, 3, 2).reshape(S, D)    # [512j+f, 128(4cq+cb)+p]
    outp = outp.reshape(1, S, D).astype(np.float32)
    if _trace:
        _CACHE["last_results"] = res
    return outp



# revision 50
# speedup vs baseline: 1.1902x; 1.1902x over previous
"""Trainium2 Bass kernel for nn_AttnAdapter: GQA attention with RoPE,
region-based enhance/suppress score scaling, causal mask, o_proj.

Sharding: tensor-parallel over heads across 8 NeuronCores. Core d holds
q-heads 4d..4d+3 (wq rows), kv-head d (wk/wv rows), and wo columns
512d..512(d+1). Each core computes a full [S, D] partial of the output;
the host sums the 8 partials (the TP all-reduce, done at unshard time).

v3: all matmuls bf16 (fp32 PSUM accumulate), weights resident in SBUF.
Phase A (projections, j order 0,2,3,1) runs m-major with k/v passes
first so acc evictions drain inside the matmul phase; RoPE needs no PE
matmul or PSUM — rotate_half is two SBUF->SBUF partition-swap DMAs with
the sign folded into sinF. Attention region E (blocks 2,3) uses wide
[128,1024] score pairs (one exp per two key tiles); region L (blocks
1,0) interleaves o_proj chunks between the scores prefetch and the
dn/av tail. The softmax denominator accumulates via an all-ones
[128,128] stationary so it lands pre-broadcast across partitions
(1/x = exp(-ln x) on ACT, or DVE reciprocal where ACT is the pacer).
Inputs stream over three DMA queues (SP: x, ACT: weights+consts,
SWDGE: half the outputs).
"""

import math

import numpy as np

# ---- problem constants (hardcoded; kernel.py must be self-contained) ----
S = 2048          # sequence length
D = 4096          # model dim
HD = 128          # head dim
NCORES = 8
QH = 4            # q heads per core
SYS_LEN, IMG_LEN = 35, 576
BOUND = SYS_LEN + IMG_LEN          # 611
ENH, SUP = 1.5, 0.5
ROPE_BASE = 10000.0

J = 4             # sq tiles of 512
NSK = 16          # sk tiles of 128
DCH = 32          # D chunks of 128
G = 4             # d-groups of 8 chunks
GD = 8            # d-chunks per group

_CACHE = {}


def _bf16():
    import ml_dtypes
    return ml_dtypes.bfloat16


def _host_constants():
    bf16 = _bf16()
    inv_freq = 1.0 / (ROPE_BASE ** (np.arange(0, HD, 2, dtype=np.float32) / HD))
    pos = np.arange(S, dtype=np.float32)
    freqs = pos[:, None] * inv_freq[None, :]              # [S, 64]
    emb = np.concatenate([freqs, freqs], axis=-1)         # [S, 128]
    cosT = np.ascontiguousarray(np.cos(emb).T.astype(np.float32))  # [128, S]
    # rotate_half is a partition swap plus a sign flip: rot(q)[0:64] =
    # -q[64:128], rot(q)[64:128] = q[0:64]. The swap rides two SBUF->SBUF
    # DMAs and the sign lives in sinF (rows 0:64 negated), so no PE matmul
    # or PSUM is needed for RoPE at all.
    sinF = np.sin(emb).T.astype(np.float32)
    sinF[:HD // 2] = -sinF[:HD // 2]
    sinF = np.ascontiguousarray(sinF)

    ident = np.eye(HD, dtype=np.float32)

    # Diagonal-tile causal masks, T layout [sk 128, sq 512]:
    # tile (i=4j+delta, j): valid (keep) iff sq >= sk  <=>  f >= 128*delta + p
    masks = np.zeros((HD, 4 * 512), dtype=np.float32)
    p = np.arange(128)[:, None]
    f = np.arange(512)[None, :]
    for delta in range(4):
        masks[:, delta * 512:(delta + 1) * 512] = (f >= 128 * delta + p)

    kpos = np.arange(S)
    key_scale = np.where(kpos < SYS_LEN, SUP,
                         np.where(kpos < BOUND, ENH, 1.0)).astype(np.float32)
    # per-partition scale per sk-tile (j=1 partial-row DVE path)
    ksT = np.ascontiguousarray(key_scale[:5 * 128].reshape(5, 128).T)  # [128, 5]
    # per-key-column broadcast (for pre-scaling krot columns, j>=2 path)
    ksB = np.ascontiguousarray(
        np.broadcast_to(key_scale[:640][None, :], (HD, 640)))          # [128, 640]

    return dict(
        cosT=cosT, sinF=sinF,
        ident=ident.astype(bf16),
        masks=masks.astype(bf16), ksT=ksT, ksB=ksB.astype(bf16),
        ones_mat=np.ones((HD, HD), dtype=bf16),
    )


def _build_bass():
    import concourse.bass as bass
    import concourse.mybir as mybir
    from concourse.tile import TileContext
    from contextlib import ExitStack

    f32 = mybir.dt.float32
    f32r = mybir.dt.float32r
    bf = mybir.dt.bfloat16

    nc = bass.Bass()
    # x packed per (j, g): [128, GD*512] with [p, dd*512+f] = x.T[128(8g+dd)+p, 512j+f]
    xp = nc.dram_tensor("xp", [J, G, 128, GD * 512], bf, kind="ExternalInput")
    # wq packed per g: [128, GD*512] with [p, dd*512+f] = wq_d.T[128(8g+dd)+p, f]
    wqp = nc.dram_tensor("wqp", [G, 128, GD * 512], bf, kind="ExternalInput")
    # wk/wv packed per g: [128, GD*256], cols dd*256+0:128 = wkT, 128:256 = wvT
    wkvp = nc.dram_tensor("wkvp", [G, 128, GD * 256], bf, kind="ExternalInput")
    # wo packed: [128, 4*4096] with [p, h*4096+f] = wo[:, core].T[128h+p, f]
    wop = nc.dram_tensor("wop", [128, QH * D], bf, kind="ExternalInput")
    cosT_d = nc.dram_tensor("cosT", [HD, S], f32, kind="ExternalInput")
    sinF_d = nc.dram_tensor("sinF", [HD, S], f32, kind="ExternalInput")
    ident_d = nc.dram_tensor("ident", [HD, HD], bf, kind="ExternalInput")
    masks_d = nc.dram_tensor("masks", [HD, 4 * 512], bf, kind="ExternalInput")
    ksT_d = nc.dram_tensor("ksT", [HD, 5], f32, kind="ExternalInput")
    ksB_d = nc.dram_tensor("ksB", [HD, 640], bf, kind="ExternalInput")
    onesm_d = nc.dram_tensor("ones_mat", [HD, HD], bf, kind="ExternalInput")
    # out.T tiles: out_t[j, cq, p, cb*512+f] = out.T[128*(4cq+cb)+p, 512j+f]
    out = nc.dram_tensor("out", [J, DCH // 4, 128, 4 * 512], bf,
                         kind="ExternalOutput")

    EXP = mybir.ActivationFunctionType.Exp

    with TileContext(nc) as tc, ExitStack() as ctx:
        # ---- resident weights (single load, used all phases) ----
        # Order matters for startup latency: the first matmuls need only
        # wq/wkv chunk g=0 and x(j=0,g=0); everything else trails behind.
        wsb = ctx.enter_context(tc.tile_pool(name="wsb", bufs=1))
        wq_sb = wsb.tile([128, DCH * 512], bf)
        wkv_sb = wsb.tile([128, DCH * 256], bf)
        xgp = ctx.enter_context(tc.tile_pool(name="xg", bufs=3))
        xpre = {}
        xt0 = xgp.tile([128, GD * 512], bf, tag="xt", name="xt0")
        # Fine-grained first slices so the first matmuls (which need only
        # dd=0 of g=0) start after ~320KB of DMA instead of ~2MB, spread
        # over the two fast HWDGE queues: wq+x interleaved on SP, wkv (and
        # later the consts) on ACT. The gpsimd SWDGE queue measures only
        # ~40-70 GB/s, so it carries no phase-A-critical traffic.
        # 2+3 slices per stream: 512-col for dd 0-1 (smallest first dep),
        # 1024-col after — few enough descriptors that the ~600ns-per-
        # descriptor programming on the queues keeps ahead of consumption
        cuts = [0, 512, 1024, 2048, 3072, 4096]
        for a, b_ in zip(cuts, cuts[1:]):
            nc.sync.dma_start(wq_sb[:, a:b_], wqp[0][:, a:b_])
            nc.sync.dma_start(xt0[:, a:b_], xp[0, 0][:, a:b_])
            nc.scalar.dma_start(wkv_sb[:, a // 2:b_ // 2],
                                wkvp[0][:, a // 2:b_ // 2])
        xpre[(0, 0)] = xt0
        for g in range(1, G):
            # wq g1-3 rides the ACT queue: the SP queue is reserved for the
            # x stream (16.8MB) so x tiles never queue behind weights
            nc.scalar.dma_start(wq_sb[:, g * GD * 512:(g + 1) * GD * 512],
                                wqp[g])
            nc.scalar.dma_start(wkv_sb[:, g * GD * 256:(g + 1) * GD * 256],
                                wkvp[g])
            if g <= 2:
                xt_n = xgp.tile([128, GD * 512], bf, tag="xt", name=f"xt{g}")
                nc.sync.dma_start(xt_n[:], xp[0, g])
                xpre[(0, g)] = xt_n

        # const tiles allocated here; their DMAs are deferred until after
        # j=0's x tiles are queued so phase A's first block never starves
        const = ctx.enter_context(tc.tile_pool(name="const", bufs=1))
        cosT = const.tile([HD, S], f32)
        sinF = const.tile([HD, S], f32)
        ident = const.tile([HD, HD], bf)
        masks = const.tile([HD, 4 * 512], bf)
        ksT = const.tile([HD, 5], f32)
        ksB = const.tile([HD, 640], bf)
        ones_mat = const.tile([HD, HD], bf)
        wo_sb = const.tile([128, QH * D], bf)

        def emit_const_dmas():
            # consts ride the ACT HWDGE queue (done programming its wkv
            # slices by then, and ACT's compute work starts much later)
            nc.scalar.dma_start(cosT[:], cosT_d[:, :])
            nc.scalar.dma_start(sinF[:], sinF_d[:, :])
            nc.scalar.dma_start(ident[:], ident_d[:, :])
            nc.scalar.dma_start(masks[:], masks_d[:, :])
            nc.scalar.dma_start(ksT[:], ksT_d[:, :])
            nc.scalar.dma_start(ksB[:], ksB_d[:, :])
            nc.scalar.dma_start(ones_mat[:], onesm_d[:, :])
            nc.scalar.dma_start(wo_sb[:], wop[:, :])

        # j=1's q-rope intermediates live here (not in phase A's stage pool)
        # so closing the A pools never waits on the deferred GPSIMD chain
        ropeL = ctx.enter_context(tc.tile_pool(name="ropeL", bufs=2))
        persist = ctx.enter_context(tc.tile_pool(name="persist", bufs=1))
        qrot = [persist.tile([HD, S], bf, name=f"qrot{m}") for m in range(QH)]
        krot = persist.tile([HD, S], bf)
        krot_s = persist.tile([HD, 640], bf)     # region-scaled K cols (keys<640)
        vnat = persist.tile([HD, NSK * HD], bf)  # tile i at cols i*128
        attn = [persist.tile([HD, S], bf, name=f"attn{h}") for h in range(QH)]

        # ---------------- Phase A: projections + RoPE + V transpose --------
        # Phase A runs j in [0, 2, 3, 1] so region E's blocks (2, 3) get
        # their RoPE outputs mid-A; the last epilogue (j=1) feeds only
        # region L, ~70us later.
        with tc.tile_pool(name="accp", bufs=1, space="PSUM") as accp, \
             tc.tile_pool(name="vtp", bufs=2, space="PSUM") as vtp, \
             tc.tile_pool(name="stage", bufs=2) as stage:
            for j in (0, 2, 3, 1):
                sq = slice(j * 512, (j + 1) * 512)
                accs = [accp.tile([128, 512], f32, name=f"acc{m}") for m in range(6)]
                for g in range(G):
                    if (j, g) in xpre:
                        xt = xpre.pop((j, g))
                    else:
                        xt = xgp.tile([128, GD * 512], bf, tag="xt")
                        nc.sync.dma_start(xt[:], xp[j, g])
                        if j == 0 and g == G - 1:
                            emit_const_dmas()
                    def emit_proj(m, dd):
                        d = g * GD + dd
                        xs = xt[:, dd * 512:(dd + 1) * 512]
                        st = (d == 0)
                        sp_ = (d == DCH - 1)
                        if m < QH:
                            w = wq_sb[:, d * 512 + m * 128:d * 512 + (m + 1) * 128]
                            nc.tensor.matmul(accs[m][:], w, xs,
                                             start=st, stop=sp_)
                        else:
                            kb = d * 256 + (m - QH) * 128
                            nc.tensor.matmul(accs[m][:], wkv_sb[:, kb:kb + 128],
                                             xs, start=st, stop=sp_)

                    if j == 0 and g == 0:
                        # dd-major: consumption tracks the fine-grained
                        # lead-in DMA slices
                        for dd in range(GD):
                            for m in range(6):
                                emit_proj(m, dd)
                    else:
                        # m-major within the group, k/v passes first: each
                        # acc's accumulation stops progressively earlier than
                        # the j boundary, matching the epilogue's eviction
                        # order (k, v, q0..q3), so the copies drain inside
                        # the matmul phase instead of stalling PE at the
                        # boundary
                        for m in (4, 5, 0, 1, 2, 3):
                            for dd in range(GD):
                                emit_proj(m, dd)

                # RoPE epilogue: evict acc to f32 SBUF (full precision), swap
                # halves via two SBUF->SBUF DMAs, then dst = q*cos + swap*sinF
                # on DVE/GPSIMD. No PE matmul, no PSUM bank. The swap DMAs
                # ride the ACT HWDGE queue (its const/weight traffic is done
                # by the first epilogue); the j=1 k-rope uses the SP queue
                # (idle at A end) since region E's pair p2 needs it soonest.
                def emit_rope(m, copy_eng, mul_eng, dma_eng):
                    dst = qrot[m][:, sq] if m < QH else krot[:, sq]
                    q_sb = stage.tile([128, 512], f32, tag="q_sb", bufs=2)
                    if copy_eng == "act":
                        nc.scalar.copy(q_sb[:], accs[m][:])
                    else:
                        nc.vector.tensor_copy(q_sb[:], accs[m][:])
                    rp = stage.tile([128, 512], f32, tag="rp", bufs=2)
                    dma_eng.dma_start(rp[0:64, :], q_sb[64:128, :])
                    dma_eng.dma_start(rp[64:128, :], q_sb[0:64, :])
                    eng = nc.vector if mul_eng == "dve" else nc.gpsimd
                    t1 = stage.tile([128, 512], f32, tag="t1", bufs=2)
                    eng.tensor_mul(t1[:], q_sb[:], cosT[:, sq])
                    t2 = stage.tile([128, 512], f32, tag="t2", bufs=2)
                    eng.tensor_mul(t2[:], rp[:], sinF[:, sq])
                    eng.tensor_add(dst, t1[:], t2[:])

                def emit_vtrans():
                    v_sb = stage.tile([128, 512], bf, tag="v_sb")
                    nc.scalar.copy(v_sb[:], accs[5][:])
                    vt_ps = vtp.tile([128, 512], bf, tag="vt_ps")
                    for b in range(4):
                        nc.tensor.transpose(vt_ps[:, b * 128:(b + 1) * 128],
                                            v_sb[:, b * 128:(b + 1) * 128],
                                            ident[:])
                    nc.vector.tensor_copy(
                        vnat[:, (4 * j) * 128:(4 * j + 4) * 128], vt_ps[:])

                if j != 1:
                    emit_rope(4, "act", "dve", nc.scalar)
                    emit_vtrans()
                    for m in range(QH):
                        emit_rope(m, "act", "dve", nc.scalar)
                    if j == 0:
                        nc.vector.tensor_mul(krot_s[:, 0:512], krot[:, 0:512],
                                             ksB[:, 0:512])
                else:
                    # Last epilogue (j=1): all six acc evictions go FIRST
                    # (1 ACT + 3 DVE q-copies, then v + k on ACT) so region
                    # E's PSUM pools, which reuse these banks, unblock within
                    # ~2us. k's rope chain stays on DVE (E's pair p2 needs
                    # krot tiles 4-7 in ~5us); the q ropes run on GPSIMD with
                    # tiles from ropeL — their only consumer is region L.
                    q_sbs = {}
                    for m, ceng in ((0, "act"), (1, "dve"),
                                    (2, "dve"), (3, "dve")):
                        qs = ropeL.tile([128, 512], f32, tag=f"q{m}", bufs=1,
                                        name=f"q_sb1_{m}")
                        if ceng == "act":
                            nc.scalar.copy(qs[:], accs[m][:])
                        else:
                            nc.vector.tensor_copy(qs[:], accs[m][:])
                        q_sbs[m] = qs
                    emit_vtrans()
                    emit_rope(4, "act", "dve", nc.sync)
                    nc.vector.tensor_mul(krot_s[:, 512:640], krot[:, 512:640],
                                         ksB[:, 512:640])
                    for m in range(QH):
                        dst = qrot[m][:, sq]
                        qs = q_sbs[m]
                        rp = ropeL.tile([128, 512], f32, tag="rp", bufs=1, name="rp1")
                        nc.scalar.dma_start(rp[0:64, :], qs[64:128, :])
                        nc.scalar.dma_start(rp[64:128, :], qs[0:64, :])
                        t1 = ropeL.tile([128, 512], f32, tag="t1", bufs=1, name="t11")
                        nc.gpsimd.tensor_mul(t1[:], qs[:], cosT[:, sq])
                        t2 = ropeL.tile([128, 512], f32, tag="t2", bufs=1, name="t21")
                        nc.gpsimd.tensor_mul(t2[:], rp[:], sinF[:, sq])
                        nc.gpsimd.tensor_add(dst, t1[:], t2[:])

        # ------------- Phase B+C: attention fused with o_proj --------------
        # Two regions. Region E: block j=3 alone (no o_proj available yet)
        # processed as WIDE PAIRS — two key tiles share one [128,1024] PSUM
        # scores tile and one exp instruction, halving the per-exp fixed
        # overhead that would otherwise make ACT the pacer; its finalizes
        # run on DVE (reciprocal) to stay off the ACT FIFO. Region L: blocks
        # 1, 2, 0 narrow with o_proj C chunks of finished blocks emitted
        # between the scores prefetch and the tail, so PE has independent
        # work in the exp shadow; finalizes on ACT (slack there).
        with tc.tile_pool(name="att_w", bufs=3) as att_w, \
             tc.tile_pool(name="att_sb", bufs=4) as att_sb, \
             tc.tile_pool(name="nrm", bufs=2) as nrm, \
             tc.tile_pool(name="ost", bufs=2) as ost:
            state = {}
            osb_state = {}

            def kt_for(j, i):
                # scaled K for full-region rows (j>=2), keys<640
                if j >= 2 and i < 5:
                    return krot_s[:, i * 128:(i + 1) * 128]
                return krot[:, i * 128:(i + 1) * 128]

            def emit_finalize(j, h, on_dve=False):
                sq = slice(j * 512, (j + 1) * 512)
                acc_av, acc_dn = state.pop((j, h))
                # acc_dn is already partition-broadcast (all-ones stationary)
                rb_sb = nrm.tile([128, 512], f32, tag="rb_sb")
                if on_dve:
                    nc.vector.reciprocal(rb_sb[:], acc_dn[:])
                else:
                    lrec = nrm.tile([128, 512], f32, tag="lrec")
                    nc.scalar.activation(lrec[:], acc_dn[:],
                                         mybir.ActivationFunctionType.Ln)
                    nc.scalar.activation(rb_sb[:], lrec[:], EXP, scale=-1.0)
                nc.vector.tensor_mul(attn[h][:, sq], acc_av[:], rb_sb[:])

            # ---------------- Region E: blocks 2 then 3, wide pairs --------
            with tc.tile_pool(name="spw", bufs=2, space="PSUM") as spw, \
                 tc.tile_pool(name="avpE", bufs=2, space="PSUM") as avpE, \
                 tc.tile_pool(name="dnpE", bufs=2, space="PSUM") as dnpE:
                seqE = [(jE, h, p) for jE in (2, 3) for h in range(QH)
                        for p in range((4 * jE + 4) // 2)]
                # B(2) h0's pair p2 reads krot_s[512:640], produced by the
                # j=1 k-rope chain ~4us after phase A ends; run p3 (plain
                # krot, ready ~1us earlier) ahead of it to shrink the wait
                seqE[2], seqE[3] = seqE[3], seqE[2]

                def emit_scores_w(jE, h, p):
                    sq0 = jE * 512
                    s_ps = spw.tile([128, 1024], f32, tag="sw")
                    for u in (0, 1):
                        i = 2 * p + u
                        delta = i - 4 * jE
                        c0 = 128 * delta if delta >= 1 else 0
                        nc.tensor.matmul(
                            s_ps[:, u * 512 + c0:(u + 1) * 512],
                            kt_for(jE, i),
                            qrot[h][:, sq0 + c0:sq0 + 512],
                            start=True, stop=True)
                    return s_ps

                def emit_tail_w(jE, h, p, s_ps):
                    npairs = (4 * jE + 4) // 2
                    if p == 0:
                        state[(jE, h)] = (
                            avpE.tile([128, 512], f32, tag="av", name="aE"),
                            dnpE.tile([128, 512], f32, tag="dn", name="dE"))
                    acc_av, acc_dn = state[(jE, h)]
                    e_w = att_w.tile([128, 1024], bf, tag="ew")
                    d0 = 2 * p - 4 * jE
                    if d0 >= 0:
                        # diagonal pair: per-tile exps over the live windows
                        # (the gap holds stale PSUM; exp'ing it would NaN via
                        # inf*0) + the usual 0/1 mask multiplies
                        for u in (0, 1):
                            d = d0 + u
                            c0 = 128 * d if d >= 1 else 0
                            lo = u * 512 + c0
                            nc.scalar.activation(e_w[:, lo:(u + 1) * 512],
                                                 s_ps[:, lo:(u + 1) * 512],
                                                 EXP)
                            nc.vector.tensor_mul(
                                e_w[:, lo:(u + 1) * 512],
                                e_w[:, lo:(u + 1) * 512],
                                masks[:, d * 512 + c0:(d + 1) * 512])
                    else:
                        nc.scalar.activation(e_w[:], s_ps[:], EXP)
                    for u in (0, 1):
                        i = 2 * p + u
                        d = d0 + u
                        c0 = 128 * d if d >= 1 else 0
                        st = (p == 0 and u == 0)
                        sp_l = (p == npairs - 1 and u == 1)
                        eb = e_w[:, u * 512 + c0:(u + 1) * 512]
                        nc.tensor.matmul(acc_dn[:, c0:512], ones_mat[:], eb,
                                         start=st, stop=sp_l)
                        nc.tensor.matmul(acc_av[:, c0:512],
                                         vnat[:, i * 128:(i + 1) * 128],
                                         eb, start=st, stop=sp_l)

                pendingE = None
                sw_cur = emit_scores_w(*seqE[0])
                for k, (jE, h, p) in enumerate(seqE):
                    nxt = seqE[k + 1] if k + 1 < len(seqE) else None
                    sw_next = emit_scores_w(*nxt) if nxt is not None else None
                    emit_tail_w(jE, h, p, sw_cur)
                    sw_cur = sw_next
                    if pendingE is not None and pendingE != (jE, h):
                        emit_finalize(*pendingE, on_dve=True)
                        pendingE = None
                    if p == (4 * jE + 4) // 2 - 1:
                        pendingE = (jE, h)
                # flush inside the region: the acc tiles must be consumed
                # before their PSUM banks are re-pooled by region L. ACT is
                # free at the region boundary and beats DVE's slow
                # reciprocal, so the last block's C unblocks sooner in L.
                if pendingE is not None:
                    emit_finalize(*pendingE, on_dve=False)

            # ---------------- Region L: blocks 1, 2, 0 + o_proj ------------
            # op pool first: pool->bank assignment follows creation order, so
            # op lands on region E's earliest-freed banks (spw slot A) and
            # the first C chunks don't wait on E's last finalizes
            with tc.tile_pool(name="op", bufs=2, space="PSUM") as op, \
                 tc.tile_pool(name="sp", bufs=2, space="PSUM") as sp, \
                 tc.tile_pool(name="avp", bufs=2, space="PSUM") as avp, \
                 tc.tile_pool(name="dnp", bufs=2, space="PSUM") as dnp:
                border = [1, 0]
                # C chunks interleaved: C(2) into B(1), C(3) into B(0),
                # C(1) + C(0) trailing as pure PE work
                cmap = {1: 2, 0: 3}
                # last block processes head 0 last so trailing C(0) chunks
                # (whose o_proj accumulation ends with head 0) wait minimally
                horder = {j: list(range(QH)) for j in range(J)}
                horder[border[-1]] = [1, 2, 3, 0]
                b_seq = []
                attach = {}   # index into b_seq -> list of C items
                for j in border:
                    bi = [("B", j, h, i)
                          for h in horder[j] for i in range(4 * j + 4)]
                    base = len(b_seq)
                    b_seq.extend(bi)
                    jc = cmap[j]
                    cc = [("C", jc, c) for c in range(DCH)]
                    nb = len(bi)
                    ci = 0
                    for idx in range(nb):
                        # hold C back a few items so the source block's last
                        # finalize lands before its chunks are consumed
                        while (ci < len(cc) and idx >= 3
                               and ci + 1 <= (idx - 2) * len(cc) / (nb - 3)):
                            attach.setdefault(base + idx, []).append(cc[ci])
                            ci += 1
                    attach.setdefault(base + nb - 1, []).extend(cc[ci:])
                tail_c = [("C", 1, c) for c in range(DCH)]
                tail_c += [("C", 0, c) for c in range(DCH)]
                b_next = {}
                for a, b in zip(b_seq, b_seq[1:]):
                    b_next[a] = b

                def emit_scores(el):
                    _, j, h, i = el
                    delta = i - 4 * j
                    c0 = 128 * delta if delta >= 1 else 0  # trimmed live cols
                    sq0 = j * 512
                    s_ps = sp.tile([128, 512], f32, tag="s")
                    nc.tensor.matmul(s_ps[:, c0:512], kt_for(j, i),
                                     qrot[h][:, sq0 + c0:sq0 + 512],
                                     start=True, stop=True)
                    if j == 1 and i < 5:
                        # partial region rows: queries 611.. are cols 99..
                        nc.vector.tensor_scalar_mul(
                            s_ps[:, 99:512], s_ps[:, 99:512], ksT[:, i:i + 1])
                    return s_ps

                def emit_tail(el, s_ps):
                    _, j, h, i = el
                    ni = 4 * j + 4
                    delta = i - 4 * j
                    c0 = 128 * delta if delta >= 1 else 0
                    if i == 0:
                        state[(j, h)] = (
                            avp.tile([128, 512], f32, tag="av", name="acc_av"),
                            dnp.tile([128, 512], f32, tag="dn", name="acc_dn"))
                    acc_av, acc_dn = state[(j, h)]
                    e_sb = att_sb.tile([128, 512], bf, tag="e")
                    nc.scalar.activation(e_sb[:, c0:512], s_ps[:, c0:512], EXP)
                    if delta >= 0:
                        nc.vector.tensor_mul(
                            e_sb[:, c0:512], e_sb[:, c0:512],
                            masks[:, delta * 512 + c0:(delta + 1) * 512])
                    st = (i == 0)
                    sp_l = (i == ni - 1)
                    eb = e_sb[:, c0:512]
                    # all-ones stationary: every out partition gets the
                    # key-sum, i.e. the denominator arrives pre-broadcast
                    # (same PE cost — cost is rows, not cols)
                    nc.tensor.matmul(acc_dn[:, c0:512], ones_mat[:], eb,
                                     start=st, stop=sp_l)
                    nc.tensor.matmul(acc_av[:, c0:512],
                                     vnat[:, i * 128:(i + 1) * 128],
                                     eb, start=st, stop=sp_l)

                def emit_cchunk(jc, c):
                    sq = slice(jc * 512, (jc + 1) * 512)
                    if c % 4 == 0:
                        osb_state[jc] = ost.tile([128, 4 * 512], bf,
                                                 tag="o_sb", name="o_sb")
                    o_sb = osb_state[jc]
                    cb = c % 4
                    o_ps = op.tile([128, 512], f32, tag="o")
                    for hi, h in enumerate([1, 2, 3, 0]):
                        nc.tensor.matmul(
                            o_ps[:],
                            wo_sb[:, h * D + c * 128:h * D + (c + 1) * 128],
                            attn[h][:, sq], start=(hi == 0),
                            stop=(hi == QH - 1))
                    # evictions stay off ACT: the strict-FIFO ACT queue
                    # would delay the exps that pace the pipeline
                    nc.vector.tensor_copy(
                        o_sb[:, cb * 512:(cb + 1) * 512], o_ps[:])
                    # two half-DMAs: first half on the (slow, otherwise idle)
                    # gpsimd SWDGE queue, last half on the fast SP queue so
                    # the final transfer at kernel end drains quickly
                    if cb == 1:
                        nc.gpsimd.dma_start(out[jc, c // 4][:, 0:1024],
                                            o_sb[:, 0:1024])
                    elif cb == 3:
                        nc.sync.dma_start(out[jc, c // 4][:, 1024:2048],
                                          o_sb[:, 1024:2048])

                s2_cur = emit_scores(b_seq[0])
                pending = None
                for bidx, el in enumerate(b_seq):
                    nxt = b_next.get(el)
                    s2_next = emit_scores(nxt) if nxt is not None else None
                    for cel in attach.get(bidx, ()):
                        # a C chunk reads all four attn heads of its block:
                        # any pending finalize for that block must land first
                        if pending is not None and pending[0] == cel[1]:
                            emit_finalize(*pending)
                            pending = None
                        emit_cchunk(cel[1], cel[2])
                    emit_tail(el, s2_cur)
                    s2_cur = s2_next
                    if pending is not None and pending != (el[1], el[2]):
                        emit_finalize(*pending)
                        pending = None
                    if el[3] == 4 * el[1] + 3:   # last key tile of (j, h)
                        pending = (el[1], el[2])
                for cel in tail_c:
                    if pending is not None and pending[0] == cel[1]:
                        emit_finalize(*pending)
                        pending = None
                    emit_cchunk(cel[1], cel[2])
                if pending is not None:
                    emit_finalize(*pending)

    # Split multi-wait instructions onto standalone EventSemaphore insts.
    import bass_rust
    bass_rust.generate_event_semaphores(nc)
    return nc


def _get_compiled():
    if "nc" not in _CACHE:
        _CACHE["nc"] = _build_bass()
        _CACHE["const"] = _host_constants()
    return _CACHE["nc"], _CACHE["const"]


def kernel(hidden_states, wq, wk, wv, wo, _trace=False):
    from concourse.bass_utils import run_bass_kernel_spmd

    nc, cst = _get_compiled()
    bf16 = _bf16()

    x = np.asarray(hidden_states, dtype=np.float32).reshape(S, D)
    xT = x.T                                             # [D, S]
    # xp[j, g, p, dd*512+f] = xT[128*(8g+dd)+p, 512j+f]
    t = xT.reshape(G, GD, 128, J, 512)                   # [g, dd, p, j, f]
    xpk = np.ascontiguousarray(
        t.transpose(3, 0, 2, 1, 4).reshape(J, G, 128, GD * 512)).astype(bf16)
    wq = np.asarray(wq, dtype=np.float32)
    wk = np.asarray(wk, dtype=np.float32)
    wv = np.asarray(wv, dtype=np.float32)
    wo = np.asarray(wo, dtype=np.float32)
    scale = 1.0 / math.sqrt(HD)

    in_maps = []
    for d in range(NCORES):
        wq_d = wq[d * QH * HD:(d + 1) * QH * HD] * scale      # [512, D]
        wqT = wq_d.T                                          # [4096, 512]
        wqpk = np.ascontiguousarray(
            wqT.reshape(G, GD, 128, 512).transpose(0, 2, 1, 3)
            .reshape(G, 128, GD * 512)).astype(bf16)
        wkT = wk[d * HD:(d + 1) * HD].T.reshape(G, GD, 128, 128)
        wvT = wv[d * HD:(d + 1) * HD].T.reshape(G, GD, 128, 128)
        kv = np.concatenate([wkT, wvT], axis=-1)              # [G, GD, 128, 256]
        wkvpk = np.ascontiguousarray(
            kv.transpose(0, 2, 1, 3).reshape(G, 128, GD * 256)).astype(bf16)
        woT = wo[:, d * QH * HD:(d + 1) * QH * HD].T          # [512, 4096]
        wopk = np.ascontiguousarray(
            woT.reshape(QH, 128, D).transpose(1, 0, 2)
            .reshape(128, QH * D)).astype(bf16)
        in_maps.append({
            "xp": xpk,
            "wqp": wqpk,
            "wkvp": wkvpk,
            "wop": wopk,
            "cosT": cst["cosT"], "sinF": cst["sinF"],
            "ident": cst["ident"],
            "masks": cst["masks"], "ksT": cst["ksT"], "ksB": cst["ksB"],
            "ones_mat": cst["ones_mat"],
        })

    res = run_bass_kernel_spmd(nc, in_maps, core_ids=list(range(NCORES)),
                               trace=_trace)
    acc = res.results[0]["out"].astype(np.float32)
    for d in range(1, NCORES):
        acc += res.results[d]["out"].astype(np.float32)
    # out_t[j, cq, p, cb*512+f] = out.T[128*(4cq+cb)+p, 512j+f]
    acc = acc.reshape(J, DCH // 4, 128, 4, 512)          # [j, cq, p, cb, f]
    outp = acc.transpose(0, 4, 1, 3, 2).reshape(S, D)    # [512j+f, 128(4cq+cb)+p]
    outp = outp.reshape(1, S, D).astype(np.float32)
    if _trace:
        _CACHE["last_results"] = res
    return outp

